# revision 13
# baseline (speedup 1.0000x reference)
"""Trainium2 Bass kernel for nn_LlamaQuantizedMLP (int4 fake-quant SwiGLU MLP).

Strategy (v3: fp8 stationary weights + FWL, packed PSUM banks)
--------------------------------------------------------------
Reference: per-row int4 fake quant of each weight (scale = max|w|/7,
q = clip(round(w/scale), -8, 7), w' = q*scale), then
  gate = x @ wg'.T ; up = x @ wu'.T ; h = silu(gate)*up ; y = h @ wd'.T

Int4 q values lie in [-8, 7] and are *exactly* representable in fp8 e4m3,
so the weights ship to HBM as fp8 — half the bytes of bf16, and the HBM
stream is the roofline for this problem (~17 MB/core @ ~390 GB/s).

On the PE, the fp8 weight tile is the *stationary* operand (128x128;
Fast-Weight-Load ingests fp8 at 4 elem/cycle/partition = 2x the rate of
streaming bf16 as the moving operand), while the tiny activations (x/h,
8 bf16 columns) are the moving operand.  Per-row weight scales are
applied after the matmuls (gate/up: on device before SwiGLU; down: on
host after the cross-core reduction).  This orientation produces
gate/up/h directly in [i-partition, batch] layout — exactly the rhs
layout the down matmul needs — no on-device transpose at all.

Pipelining details:
 - All weights live in one [128, 1056, 128] fp8 tensor ordered exactly
   in PE consumption order, streamed in blocks on the Sync HWDGE queue
   (issued before anything else so the first weight byte moves ASAP);
   x / scales / outputs ride the Scalar HWDGE queue.  First + last
   blocks are small so the PE starts early and drains fast.
 - Matmul accumulation groups are strictly sequential, so many groups
   share one PSUM bank at different column offsets (start=True clears
   only the has_written *bits* of the bank; completed groups' values
   survive).  DVE then reads whole banks in a handful of wide ops,
   never while the PE is writing that bank (fatal collision otherwise).

Sharding: tensor parallel over the intermediate dim (11008 = 8 x 1376,
zero-padded to 11 tiles of 128 per core).  Each core emits a partial
[4096, 8] output; the host sums the 8 partials and applies down scales.
"""

import numpy as np
import ml_dtypes

import concourse.bacc as bacc
import concourse.mybir as mybir
from concourse.tile import TileContext
from concourse import bass_utils

BF16 = mybir.dt.bfloat16
F32 = mybir.dt.float32
FP8 = mybir.dt.float8e4
NP_BF16 = ml_dtypes.bfloat16
NP_FP8 = ml_dtypes.float8_e4m3

NCORES = 8


def _block_plan(nw):
    """DMA block sizes (in 16KB weight tiles) summing to nw: small head
    so the PE starts early, small tail so the PE drains fast."""
    if nw <= 100:
        return [nw]
    plan = [16, 72]
    rem = nw - 88
    while rem > 176:
        plan.append(88)
        rem -= 88
    for s in (64, 48, 32, 16, 8, 4, 2, 1):
        while rem >= s:
            plan.append(s)
            rem -= s
    if plan[-1] > 16:          # small final block -> short PE drain
        plan[-1] -= 16
        plan.append(16)
    assert sum(plan) == nw
    return plan


class Cfg:
    def __init__(self, b=8, h=4096, i_full=11008, wbufs=4):
        assert h % 128 == 0 and i_full % NCORES == 0
        self.B = b                      # batch = moving-operand columns
        self.H = h
        self.I_FULL = i_full
        self.I_SH = i_full // NCORES    # 1376 per core
        self.IT = (self.I_SH + 127) // 128   # 11 i-tiles (padded)
        self.I_PAD = self.IT * 128      # 1408
        self.KC = h // 128              # 32 contraction chunks (gate/up)
        self.HT = h // 128              # 32 output tiles (down)
        self.NGU = self.IT * 2 * self.KC     # 704 gate/up weight tiles
        self.ND = self.IT * self.HT          # 352 down weight tiles
        self.NW = self.NGU + self.ND         # 1056 total
        self.PLAN = _block_plan(self.NW)
        self.BLKMAX = max(self.PLAN)
        self.WBUFS = min(wbufs, len(self.PLAN))
        # i-tile halves for overlapped SwiGLU staging
        self.IT_A = (self.IT + 1) // 2
        # down output chunks (PSUM banks): groups of ht tiles
        self.YCH = 4 if self.HT % 4 == 0 else 1
        self.HT_C = self.HT // self.YCH
        assert self.HT_C * self.B <= 512


FULL = Cfg()


def build(nc, cfg):
    """Per-core SPMD program (identical on all cores; data differs)."""
    B, IT, KC, HT = cfg.B, cfg.IT, cfg.KC, cfg.HT

    w_all = nc.dram_tensor("w_all", [128, cfg.NW, 128], FP8,
                           kind="ExternalInput")
    xt = nc.dram_tensor("xt", [128, KC, B], BF16, kind="ExternalInput")
    sgb = nc.dram_tensor("sgb", [128, IT * B], F32, kind="ExternalInput")
    sub = nc.dram_tensor("sub", [128, IT * B], F32, kind="ExternalInput")
    y2 = nc.dram_tensor("y2", [128, HT * B], F32, kind="ExternalOutput")

    with TileContext(nc) as tc:
        with (
            tc.tile_pool(name="xs", bufs=1) as xs_pool,
            tc.tile_pool(name="w", bufs=1) as w_pool,
            tc.tile_pool(name="act", bufs=1) as act_pool,
            tc.tile_pool(name="ps", bufs=1, space="PSUM") as ps_pool,
        ):
            # ---- ALL DMAs ride the sync HWDGE queue: the scalar/ACT
            # ring does not spread small transfers across the 16 SDMA
            # engines (they clump onto engine 0, which then straggles
            # behind on its 1/16 share of the weight stream).  Order:
            # first weight block, then x (the first matmul needs it),
            # then the rest of the weight stream; scales slot in early.
            blocks = []
            b0 = 0

            def emit_block(bi):
                nonlocal b0
                nt = cfg.PLAN[bi]
                wb = w_pool.tile([128, nt, 128], FP8, tag=f"wb{bi}",
                                 name=f"wb{bi}")
                nc.sync.dma_start(out=wb[:], in_=w_all[:, b0:b0 + nt, :])
                blocks.append((b0, nt, wb))
                b0 += nt

            emit_block(0)
            x_t = xs_pool.tile([128, KC, B], BF16, tag="x")
            nc.sync.dma_start(out=x_t[:], in_=xt[:])
            if len(cfg.PLAN) > 1:
                emit_block(1)
            sg_t = xs_pool.tile([128, IT * B], F32, tag="sg")
            nc.sync.dma_start(out=sg_t[:], in_=sgb[:])
            su_t = xs_pool.tile([128, IT * B], F32, tag="su")
            nc.sync.dma_start(out=su_t[:], in_=sub[:])
            for bi in range(2, len(cfg.PLAN)):
                emit_block(bi)

            def wtile(seq):
                for b0, nt, wb in blocks:
                    if seq < b0 + nt:
                        return wb[:, seq - b0, :]
                raise AssertionError(seq)

            h_bf = act_pool.tile([128, IT * B], BF16, tag="hbf")

            # ---------------- phase 1: gate & up ----------------
            # i-tile halves; each half packs its gate (up) groups into
            # one PSUM bank, SwiGLU of half A overlaps PE of half B.
            for ha, (i0, i1) in enumerate(
                    ((0, cfg.IT_A), (cfg.IT_A, IT))):
                nit = i1 - i0
                if nit == 0:
                    continue
                ps_g = ps_pool.tile([128, 512], F32, tag=f"g{ha}")
                ps_u = ps_pool.tile([128, 512], F32, tag=f"u{ha}")
                for it in range(i0, i1):
                    for gu, ps in ((0, ps_g), (1, ps_u)):
                        c0 = (it - i0) * B
                        for k in range(KC):
                            seq = (it * 2 + gu) * KC + k
                            nc.tensor.matmul(
                                ps[:, c0:c0 + B], wtile(seq), x_t[:, k, :],
                                start=(k == 0), stop=(k == KC - 1))
                # ---- SwiGLU for this half (reads full banks once; DVE
                # touches at most one PSUM operand per instruction)
                sl = slice(i0 * B, i1 * B)
                g_sb = act_pool.tile([128, IT * B], F32, tag="gsb")
                nc.vector.tensor_mul(out=g_sb[:, 0:nit * B],
                                     in0=ps_g[:, 0:nit * B], in1=sg_t[:, sl])
                u_sb = act_pool.tile([128, IT * B], F32, tag="usb")
                nc.vector.tensor_mul(out=u_sb[:, 0:nit * B],
                                     in0=ps_u[:, 0:nit * B], in1=su_t[:, sl])
                sig = act_pool.tile([128, IT * B], F32, tag="sig")
                nc.scalar.activation(
                    out=sig[:, 0:nit * B], in_=g_sb[:, 0:nit * B],
                    func=mybir.ActivationFunctionType.Sigmoid)
                silu = act_pool.tile([128, IT * B], F32, tag="silu")
                nc.vector.tensor_mul(out=silu[:, 0:nit * B],
                                     in0=g_sb[:, 0:nit * B],
                                     in1=sig[:, 0:nit * B])
                nc.vector.tensor_mul(out=h_bf[:, sl],
                                     in0=silu[:, 0:nit * B],
                                     in1=u_sb[:, 0:nit * B])

            # ---------------- phase 2: down ----------------
            y_sb = act_pool.tile([128, HT * B], F32, tag="ysb")
            for ch in range(cfg.YCH):
                ps_y = ps_pool.tile([128, 512], F32, tag=f"y{ch}",
                                    name=f"ps_y{ch}")
                for g in range(cfg.HT_C):
                    ht = ch * cfg.HT_C + g
                    for ik in range(IT):
                        seq = cfg.NGU + ht * IT + ik
                        nc.tensor.matmul(
                            ps_y[:, g * B:(g + 1) * B], wtile(seq),
                            h_bf[:, ik * B:(ik + 1) * B],
                            start=(ik == 0), stop=(ik == IT - 1))
                csl = slice(ch * cfg.HT_C * B, (ch + 1) * cfg.HT_C * B)
                nc.vector.tensor_copy(out=y_sb[:, csl],
                                      in_=ps_y[:, 0:cfg.HT_C * B])
                nc.sync.dma_start(out=y2[:, csl], in_=y_sb[:, csl])

    nc.compile()
    return nc


# ---------------------------------------------------------------------------
# host-side preparation
# ---------------------------------------------------------------------------

def _quant(w):
    """Reference int4 fake-quant: integer q (f32) and per-row scale."""
    w = np.asarray(w, np.float32)
    scale = (np.max(np.abs(w), axis=1, keepdims=True) /
             np.float32(7.0)).astype(np.float32)
    scale = np.maximum(scale, np.float32(np.finfo(np.float32).tiny))
    q = np.clip(np.round((w / scale).astype(np.float32)), -8.0, 7.0).astype(
        np.float32)
    return q, scale


def make_in_maps(x, w_gate, w_up, w_down, cfg):
    """Returns (in_maps for 8 cores, down-scale vector [H])."""
    B, H, IT, KC, HT = cfg.B, cfg.H, cfg.IT, cfg.KC, cfg.HT
    qg, sgf = _quant(w_gate)
    qu, suf = _quant(w_up)
    qd, sdf = _quant(w_down)

    # x: [B,1,H] f32 -> [128, KC, B] bf16  ([r,k,b] = x[b, k*128+r])
    x2 = np.asarray(x, np.float32).reshape(B, H)
    xt = np.ascontiguousarray(
        x2.T.reshape(KC, 128, B).transpose(1, 0, 2).astype(NP_BF16))

    in_maps = []
    for c in range(NCORES):
        isl = slice(c * cfg.I_SH, (c + 1) * cfg.I_SH)

        def pad_i_rows(q_sh):          # [I_SH, H] -> [I_PAD, H]
            out = np.zeros((cfg.I_PAD, H), np.float32)
            out[0:cfg.I_SH] = q_sh
            return out

        qg_sh = pad_i_rows(qg[isl])
        qu_sh = pad_i_rows(qu[isl])
        qd_sh = np.zeros((H, cfg.I_PAD), np.float32)
        qd_sh[:, 0:cfg.I_SH] = qd[:, isl]

        # gate/up tiles: [r, it, gu, k, c] = q[it*128+c, k*128+r]
        def gu_tiles(q_sh):            # [I_PAD, H] -> [128, IT, KC, 128]
            return q_sh.reshape(IT, 128, KC, 128).transpose(3, 0, 2, 1)

        wgu = np.stack([gu_tiles(qg_sh), gu_tiles(qu_sh)], axis=2)
        wgu = wgu.reshape(128, cfg.NGU, 128)
        # down tiles: [r, ht, ik, c] = qd[ht*128+c, ik*128+r]
        wd = qd_sh.reshape(HT, 128, IT, 128).transpose(3, 0, 2, 1)
        wd = wd.reshape(128, cfg.ND, 128)
        w_all = np.ascontiguousarray(
            np.concatenate([wgu, wd], axis=1).astype(NP_FP8))

        # per-row scales, broadcast over batch: [r, it*B+b] = s[it*128+r]
        def sc_b(s_col):               # [I_SH,1] -> [128, IT*B] f32
            s_pad = np.zeros((cfg.I_PAD,), np.float32)
            s_pad[0:cfg.I_SH] = s_col[:, 0]
            return np.ascontiguousarray(np.broadcast_to(
                s_pad.reshape(IT, 128, 1).transpose(1, 0, 2),
                (128, IT, B)).reshape(128, IT * B))

        in_maps.append({
            "w_all": w_all,
            "xt": xt,
            "sgb": sc_b(sgf[isl]),
            "sub": sc_b(suf[isl]),
        })
    return in_maps, sdf[:, 0]


_NC_CACHE = {}


def _get_nc(cfg):
    key = (cfg.B, cfg.H, cfg.I_FULL, cfg.WBUFS)
    if key not in _NC_CACHE:
        nc = bacc.Bacc(None, target_bir_lowering=False)
        build(nc, cfg)
        _NC_CACHE[key] = nc
    return _NC_CACHE[key]


def run(x, w_gate, w_up, w_down, cfg=FULL, **spmd_kwargs):
    """Full pipeline; returns (output [B,1,H] f32, BassKernelResults)."""
    in_maps, sd = make_in_maps(x, w_gate, w_up, w_down, cfg)
    nc = _get_nc(cfg)
    res = bass_utils.run_bass_kernel_spmd(
        nc, in_maps, core_ids=list(range(NCORES)), **spmd_kwargs)
    acc = np.zeros((128, cfg.HT * cfg.B), np.float32)
    for r in res.results:
        acc += r["y2"]
    # y2 [r, ht*B+b] = partial y[b, ht*128+r]
    y = acc.reshape(128, cfg.HT, cfg.B).transpose(2, 1, 0).reshape(
        cfg.B, cfg.H)
    y = y * sd[None, :]
    return y.reshape(cfg.B, 1, cfg.H).astype(np.float32), res


def kernel(x, w_gate, w_up, w_down):
    out, _ = run(x, w_gate, w_up, w_down)
    return out
